# revision 13
# baseline (speedup 1.0000x reference)
"""Trainium2 Bass kernel for strictly-causal RoPE self-attention (no softmax).

  out[b,h] = tril(rope(Q)@rope(Q)^T, -1) @ V    with K = Q.

Sharding: B*H = 8 independent (b,h) slices -> one per NeuronCore (pure data
parallel, no collectives). Per core: T=N=2048.

Per-core pipeline (bf16 matmul / f32 PSUM accumulate), organized to keep the
PE dense from ~10us on:
  - Inputs arrive t-column-chunked (CW=512): RoPE and stage 1 of superstep c
    need only chunk c landed, not the full 16MB.  Every elementwise operand
    is a whole [128, CW] tile at offset 0 (offset-slice DVE operands from
    packed tiles measurably slowed the whole program by ~20% in two
    revisions), so each (chunk, pair) needs 4 descriptors.  DMA descriptor
    issue costs ~0.6us on the issuing engine, so the ~129 input descriptors
    are split: Sync issues chunk0 pairs 0-3 then chunks 1-3 in priority
    order; GpSimd issues chunk0 pairs 4-7 itself before its RoPE share;
    Activation issues the 16 V-tile loads and all output stores.
  - RoPE runs column-chunked, split DVE (pairs 0-5) / GpSimd (pairs 6-7);
    Activation only evicts PSUM so it never blocks RoPE.
  - PE order: s1(0) (contraction-outer so each landed RoPE tile is consumed
    immediately), s1(1), s2(0), s1(2), s2(1), s1(3), s2(2), s2(3): emitting
    s1(c+1) before s2(c) hides pt-evict latency; the V loads land early so
    s2(0) fills the chunk-1 RoPE window.
  - Strict-causal masks of diagonal 128x128 blocks run on GpSimd, emitted
    between its RoPE chunk shares so they stay timely.
  - Output is written bf16 (half the DMA bytes); host converts to f32.
"""

import os
import sys
import math

for _p in ("/opt/trn_rl_repo", "/root/.axon_site/_ro/trn_rl_repo"):
    if os.path.isdir(_p) and _p not in sys.path:
        sys.path.append(_p)

import numpy as np
import ml_dtypes

B, H, T, N = 2, 4, 2048, 2048
THETA = 2.0 ** 16
NCORES = 8
CW = 512  # superstep width (t-columns) and output n-chunk width

bf16 = ml_dtypes.bfloat16

LAST_RESULT = None  # BassKernelResults of the most recent run (for test.py)


def build_bass(t_len=T, n_dim=N, num_devices=NCORES):
    from concourse import bacc, mybir, tile

    nc = bacc.Bacc("TRN2", target_bir_lowering=False, debug=False,
                   num_devices=num_devices)
    bf = mybir.dt.bfloat16
    f32 = mybir.dt.float32
    mult = mybir.AluOpType.mult

    nh = n_dim // 2
    kh = nh // 128           # te/to pairs (8)
    kk_n = n_dim // 128      # total contraction tiles (16)
    nb = t_len // 128        # t-blocks (16)
    ncks = t_len // CW       # supersteps / column chunks (4)
    sw = CW // 128           # t-blocks per superstep (4)
    nch = n_dim // CW        # output n-chunks (4)
    GP_PAIRS = (6, 7)        # RoPE pairs handled by GpSimd per chunk

    # chunk-major: rows [nh*c + 128*k : +128] = pair k, chunk c
    qte = nc.declare_dram_parameter("qte", [ncks * nh, CW], bf, isOutput=False)
    qto = nc.declare_dram_parameter("qto", [ncks * nh, CW], bf, isOutput=False)
    cosd = nc.declare_dram_parameter("cosT", [ncks * nh, CW], bf, isOutput=False)
    sind = nc.declare_dram_parameter("sinT", [ncks * nh, CW], bf, isOutput=False)
    vin = nc.declare_dram_parameter("v", [t_len, n_dim], bf, isOutput=False)
    maskd = nc.declare_dram_parameter("mask", [128, 128], bf, isOutput=False)
    outd = nc.declare_dram_parameter("out", [t_len, n_dim], bf, isOutput=True)

    # PE consumes contraction tiles in RoPE production order: DVE pairs'
    # E halves, then their O halves, then the GpSimd pairs.
    dve_pairs = [k for k in range(kh) if k not in GP_PAIRS]
    kk_order = ([k for k in dve_pairs] + [kh + k for k in dve_pairs]
                + [k for k in GP_PAIRS] + [kh + k for k in GP_PAIRS])

    with tile.TileContext(nc) as tc:
        with (
            tc.tile_pool(name="qrt", bufs=kk_n * ncks) as qrt_pool,
            tc.tile_pool(name="inp", bufs=30) as in_pool,
            tc.tile_pool(name="vres", bufs=nb) as v_pool,
            tc.tile_pool(name="tmpv", bufs=4) as tmpv_pool,
            tc.tile_pool(name="tmpg", bufs=4) as tmpg_pool,
            tc.tile_pool(name="ptile", bufs=28) as p_pool,
            tc.tile_pool(name="osb", bufs=12) as out_pool,
            tc.tile_pool(name="mk", bufs=1) as mk_pool,
            tc.tile_pool(name="psum", bufs=8, space="PSUM") as psum_pool,
        ):
            mask_sb = mk_pool.tile([128, 128], bf)

            # qrt[kk][c] tiles [128, CW]
            qrt = [[None] * ncks for _ in range(kk_n)]
            v_tiles = [None] * nb
            pend_mask = {c: [] for c in range(ncks)}
            chunk_tiles = {c: {} for c in range(ncks)}

            def dma_pairs(c, pairs, eng):
                for k in pairs:
                    r = slice(nh * c + 128 * k, nh * c + 128 * (k + 1))
                    te = in_pool.tile([128, CW], bf, tag="inp",
                                      name=f"te_{c}_{k}")
                    to = in_pool.tile([128, CW], bf, tag="inp",
                                      name=f"to_{c}_{k}")
                    ct = in_pool.tile([128, CW], bf, tag="inp",
                                      name=f"ct_{c}_{k}")
                    st = in_pool.tile([128, CW], bf, tag="inp",
                                      name=f"st_{c}_{k}")
                    eng.dma_start(te[:], qte[r, :])
                    eng.dma_start(to[:], qto[r, :])
                    eng.dma_start(ct[:], cosd[r, :])
                    eng.dma_start(st[:], sind[r, :])
                    chunk_tiles[c][k] = (te, to, ct, st)

            def rope_item(eng, tmp_pool, c, k):
                te, to, ct, st = chunk_tiles[c][k]
                qe = qrt_pool.tile([128, CW], bf, tag="qrt",
                                   name=f"qe_{k}_{c}")
                qo = qrt_pool.tile([128, CW], bf, tag="qrt",
                                   name=f"qo_{k}_{c}")
                x1 = tmp_pool.tile([128, CW], bf, tag="tmp")
                x2 = tmp_pool.tile([128, CW], bf, tag="tmp")
                eng.tensor_mul(x1[:], to[:], st[:])   # O*S
                eng.tensor_mul(qe[:], te[:], ct[:])   # E*C
                eng.tensor_sub(qe[:], qe[:], x1[:])   # E' = E*C - O*S
                eng.tensor_mul(x2[:], te[:], st[:])   # E*S
                eng.tensor_mul(qo[:], to[:], ct[:])   # O*C
                eng.tensor_add(qo[:], qo[:], x2[:])   # O' = O*C + E*S
                qrt[k][c] = qe
                qrt[kh + k][c] = qo

            def load_v(jlo, jhi, eng):
                for jb in range(jlo, min(jhi, nb)):
                    vt = v_pool.tile([128, n_dim], bf, tag="vt",
                                     name=f"v_{jb}")
                    eng.dma_start(vt[:], vin[128 * jb:128 * (jb + 1), :])
                    v_tiles[jb] = vt

            def rope_dve(c):
                for k in dve_pairs:
                    rope_item(nc.vector, tmpv_pool, c, k)

            def rope_gp(c):
                for k in GP_PAIRS:
                    rope_item(nc.gpsimd, tmpg_pool, c, k)

            def gp_masks(c):
                for pt in pend_mask[c]:
                    nc.gpsimd.tensor_tensor(pt[:, 0:128], pt[:, 0:128],
                                            mask_sb[:], mult)

            def stage1(c, outer):
                t0 = CW * c
                ptiles = {}
                chains = []
                for j in range(sw * c + sw):
                    rj0 = max(128 * j, t0)
                    w = CW * (c + 1) - rj0
                    ps = psum_pool.tile([128, w], f32, tag="psum",
                                        name=f"ps_{c}_{j}")
                    chains.append((j, rj0, w, ps))

                def emit_mm(kk, j, rj0, w, ps, ki):
                    cj, oj = divmod(j, sw)
                    nc.tensor.matmul(
                        ps[:, :],
                        qrt[kk][cj][:, 128 * oj:128 * oj + 128],
                        qrt[kk][c][:, rj0 - t0:rj0 - t0 + w],
                        start=(ki == 0), stop=(ki == kk_n - 1))

                if outer:  # contraction-outer: all chains advance per kk
                    for ki, kk in enumerate(kk_order):
                        for j, rj0, w, ps in chains:
                            emit_mm(kk, j, rj0, w, ps, ki)
                else:
                    for j, rj0, w, ps in chains:
                        for ki, kk in enumerate(kk_order):
                            emit_mm(kk, j, rj0, w, ps, ki)
                for j, rj0, w, ps in chains:
                    pt = p_pool.tile([128, w], bf, tag="pt",
                                     name=f"pt_{c}_{j}")
                    nc.scalar.copy(pt[:, :], ps[:, :])
                    if rj0 == 128 * j:   # diagonal block: strict-causal mask
                        pend_mask[c].append(pt)
                    ptiles[j] = (pt, rj0)
                return ptiles

            def stage2(c, ptiles):
                for d in range(sw):
                    i = sw * c + d
                    ti = 128 * i
                    for ch in range(nch):
                        ops = psum_pool.tile([128, CW], f32, tag="psum",
                                             name=f"ps2_{i}_{ch}")
                        for j in range(i + 1):
                            pt, rj0 = ptiles[j]
                            off = ti - rj0
                            nc.tensor.matmul(
                                ops[:, :], pt[:, off:off + 128],
                                v_tiles[j][:, CW * ch:CW * (ch + 1)],
                                start=(j == 0), stop=(j == i))
                        osb = out_pool.tile([128, CW], bf, tag="osb",
                                            name=f"osb_{i}_{ch}")
                        nc.scalar.copy(osb[:], ops[:])
                        # store issued from Sync: its input issues are done
                        # by the first osb eviction, and Act must stay free
                        # to turn PSUM banks around during stage-2 bursts
                        nc.sync.dma_start(
                            outd[ti:ti + 128, CW * ch:CW * (ch + 1)], osb[:])

            # ---- DMA issue plan ----
            # V loads are deferred into Act's evict stream so the 8MB of V
            # never contends with chunk 0/1 input landing.
            dma_pairs(0, range(0, 4), nc.sync)      # sync: highest priority
            dma_pairs(0, range(4, 8), nc.gpsimd)    # gp issues its own pairs
            dma_pairs(1, range(4, 8), nc.scalar)    # act is idle up front
            dma_pairs(1, range(0, 4), nc.sync)
            nc.sync.dma_start(mask_sb[:], maskd[:])
            dma_pairs(2, range(0, 8), nc.sync)
            dma_pairs(3, range(0, 8), nc.sync)

            # ---- per-engine emission schedule ----
            rope_gp(0)
            rope_dve(0)
            pts0 = stage1(0, outer=True)
            load_v(0, 4, nc.scalar)     # after ss0 pt evicts in Act stream
            gp_masks(0)
            rope_dve(1)
            rope_gp(1)
            pts1 = stage1(1, outer=False)
            rope_dve(2)
            rope_gp(2)
            stage2(0, pts0)
            load_v(4, 8, nc.scalar)
            gp_masks(1)
            pts2 = stage1(2, outer=False)
            rope_dve(3)
            rope_gp(3)
            stage2(1, pts1)
            load_v(8, 12, nc.scalar)
            pts3 = stage1(3, outer=False)
            load_v(12, 16, nc.scalar)
            gp_masks(2)
            stage2(2, pts2)
            gp_masks(3)
            stage2(3, pts3)

    nc.compile()
    return nc


def _tables(t_len=T, n_dim=N):
    t = np.arange(n_dim, dtype=np.float32)
    q = np.floor(t / 2.0) * 2.0
    f = (1.0 / THETA ** (q.astype(np.float64) / n_dim)
         / (2.0 * math.pi)).astype(np.float32)
    phases = np.arange(t_len, dtype=np.float32)[:, None] * f[None, :]
    ph = (phases % 1.0) * np.float32(2.0 * math.pi)
    ct = np.ascontiguousarray(np.cos(ph)[:, 0::2].T).astype(bf16)  # [N/2, T]
    st = np.ascontiguousarray(np.sin(ph)[:, 0::2].T).astype(bf16)
    return ct, st


def _chunk_major(x):
    # [nh, T] -> [ncks*nh, CW] with rows [nh*c : nh*(c+1)] = columns chunk c
    nh = x.shape[0]
    ncks = x.shape[1] // CW
    return np.ascontiguousarray(
        x.reshape(nh, ncks, CW).transpose(1, 0, 2).reshape(ncks * nh, CW))


def _mask128():
    s = np.arange(128)[:, None]
    tt = np.arange(128)[None, :]
    return (s < tt).astype(bf16)


_compiled = {}


def _get_nc():
    if "nc" not in _compiled:
        _compiled["nc"] = build_bass()
    return _compiled["nc"]


def kernel(Q, V):
    global LAST_RESULT
    from concourse.bass_utils import run_bass_kernel_spmd

    Q = np.asarray(Q)
    V = np.asarray(V)
    assert Q.shape == (B, H, T, N) and V.shape == (B, H, T, N)

    nc = _get_nc()
    ct, st = _tables()
    ctc, stc = _chunk_major(ct), _chunk_major(st)
    mask = _mask128()

    in_maps = []
    for b in range(B):
        for h in range(H):
            qs = Q[b, h]
            in_maps.append({
                "qte": _chunk_major(
                    np.ascontiguousarray(qs[:, 0::2].T).astype(bf16)),
                "qto": _chunk_major(
                    np.ascontiguousarray(qs[:, 1::2].T).astype(bf16)),
                "cosT": ctc,
                "sinT": stc,
                "v": V[b, h].astype(bf16),
                "mask": mask,
            })

    res = run_bass_kernel_spmd(nc, in_maps, core_ids=list(range(NCORES)))
    LAST_RESULT = res

    out = np.empty((B, H, T, N), dtype=np.float32)
    for b in range(B):
        for h in range(H):
            out[b, h] = res.results[b * H + h]["out"].astype(np.float32)
    return out
